# revision 85
# baseline (speedup 1.0000x reference)
"""Trainium2 Bass kernel for nn_HCMGNNBasedMetaPathModel.

Strategy: bacteria nodes sharded over 8 cores (3750 -> padded 3840 rows each);
trait side + weights replicated. Edge-list segment ops are reformulated as
dense (deg-normalized) adjacency matmuls built on the host in bf16.
Per layer the trait-side partial aggregates are combined with a bf16
ReduceScatter (which also carries a 1/8-scaled copy of the core's own xtT
block so the sharded trait update needs no rank-dependent addressing) and a
bf16 AllGather rebuilds xt; the trait update runs 2 tiles/core.
Aggregations over the adjacency run d-major (stationary = node-major
features, moving = the streamed adjacency tile, n=512) with the lin_r branch
and all biases folded into the same PSUM accumulation group (biases enter as
k=1 matmuls against a ones row); PE transposes then restore node-major tiles
for the LayerNorm/L2 epilogues, which read PSUM directly and use
bn_stats/bn_aggr for the stats. Linear-algebra folds:
  tb@Wr.T = xb@(Wr@Wt).T + (Wr@bt)      (lin_r branch)
  ttl     = xt@(Wl_b@Wt_t).T + Wl_b@bt_t (pre-multiplied neighbor features)
  mean scaling folded into adjacency rows (host).
The final phase runs trait-side work sharded (own block recovered from the
layer-2 trait update, AllGather of the normalized trait projections), then
three pipelined bacteria passes (metapath d-major agg+LN, proj1+LN+relu,
proj2+l2+sim) with whole-tile f16 similarity writes.
"""
import contextlib
import sys

for _p in ("/opt/trn_rl_repo",):
    if _p not in sys.path:
        sys.path.insert(0, _p)

import numpy as np
import ml_dtypes

import concourse.bass as bass
import concourse.tile as tile
from concourse import bacc, mybir
from concourse.bass_utils import run_bass_kernel_spmd

BF16 = ml_dtypes.bfloat16
F32 = mybir.dt.float32
F16 = mybir.dt.float16
BF = mybir.dt.bfloat16
AF = mybir.ActivationFunctionType
ALU = mybir.AluOpType
ts, ds = bass.ts, bass.ds

N_B, N_T, D, L, M = 30000, 2000, 256, 3, 2
NC = 8
B_SH = 3750          # real bacteria rows per core
BP = 3840            # padded bacteria rows per core
NBT = BP // 128      # 30 node tiles
TP = 2048            # padded trait rows
NTT = TP // 128      # 16 trait tiles
TB = TP // NC        # 256 traits per core (2 tiles) for the sharded update
LN_EPS = 1e-5
NG = (NBT + 3) // 4  # node tile groups of 4

# ---------------------------------------------------------------------------
# Host-side preprocessing
# ---------------------------------------------------------------------------

def _dense_adj(src, dst, n_dst, n_src):
    """A[d, s] = #edges (s->d), rows scaled by 1/max(deg,1)."""
    idx = dst.astype(np.int64) * n_src + src.astype(np.int64)
    A = np.bincount(idx, minlength=n_dst * n_src).astype(np.float32)
    A = A.reshape(n_dst, n_src)
    deg = np.bincount(dst.astype(np.int64), minlength=n_dst).astype(np.float32)
    A *= (1.0 / np.maximum(deg, 1.0))[:, None]
    return A


def _pmajor(tiles):
    """[n, 128, D] tile-major -> [128, n*D] partition-major (big DMA rows)."""
    return np.ascontiguousarray(np.transpose(np.asarray(tiles), (1, 0, 2))
                                ).reshape(128, -1)


def _prep(inp):
    f32 = np.float32
    emb_b = np.asarray(inp["emb_b"], f32)
    emb_t = np.asarray(inp["emb_t"], f32)

    A_tb = _dense_adj(np.asarray(inp["src_tb"]), np.asarray(inp["dst_tb"]), N_B, N_T)
    A_bt = _dense_adj(np.asarray(inp["src_bt"]), np.asarray(inp["dst_bt"]), N_T, N_B)
    mpw = np.asarray(inp["mp_w"], np.float64)
    e = np.exp(mpw - mpw.max())
    w = e / e.sum()
    sw = float(w.sum())
    mp_adj = np.asarray(inp["mp_adj"], f32)
    A_mp = (w[0] * mp_adj[0].astype(np.float64) +
            w[1] * mp_adj[1].astype(np.float64)).astype(f32)

    xb0 = np.zeros((NC, NBT, 128, D), BF16)
    xb0.reshape(NC, BP, D)[:, :B_SH] = emb_b.reshape(NC, B_SH, D).astype(BF16)
    xt0 = np.zeros((NTT, 128, D), BF16)
    xt0.reshape(TP, D)[:N_T] = emb_t.astype(BF16)
    xt0_pm = _pmajor(xt0)
    # per-core own trait block, d-major [128 dlow, 2 dhigh, 256 t]
    xt0_full = np.zeros((TP, D), BF16)
    xt0_full[:N_T] = emb_t.astype(BF16)
    xtO0 = np.zeros((NC, 128, 2, TB), BF16)
    for c in range(NC):
        blk = xt0_full[c * TB:(c + 1) * TB]  # [256 t, 256 d]
        xtO0[c] = blk.T.reshape(2, 128, TB).transpose(1, 0, 2)

    def shard_T(A):  # [N_B, N_T] -> per-core [NTT, 128, BP] trait-major bf16
        out = np.zeros((NC, NTT, 128, BP), BF16)
        for c in range(NC):
            blk = np.zeros((TP, BP), f32)
            blk[:N_T, :B_SH] = A[c * B_SH:(c + 1) * B_SH].T
            out[c] = blk.reshape(NTT, 128, BP).astype(BF16)
        return out

    At = shard_T(A_tb)
    Amp = shard_T(A_mp)
    Abt = np.zeros((NC, NBT, 128, TP), BF16)
    for c in range(NC):
        blk = np.zeros((BP, TP), f32)
        blk[:B_SH, :N_T] = A_bt[:, c * B_SH:(c + 1) * B_SH].T
        Abt[c] = blk.reshape(NBT, 128, TP).astype(BF16)

    Wt_b, bt_b = np.asarray(inp["Wt_b"], f32), np.asarray(inp["bt_b"], f32)
    Wt_t, bt_t = np.asarray(inp["Wt_t"], f32), np.asarray(inp["bt_t"], f32)
    Wl_b, bl_b = np.asarray(inp["Wl_b"], f32), np.asarray(inp["bl_b"], f32)
    Wr_b = np.asarray(inp["Wr_b"], f32)
    Wl_t, bl_t = np.asarray(inp["Wl_t"], f32), np.asarray(inp["bl_t"], f32)
    Wr_t = np.asarray(inp["Wr_t"], f32)
    lng_b = np.asarray(inp["lng_b"], f32)
    lnb_b = np.asarray(inp["lnb_b"], f32)
    lng_t = np.asarray(inp["lng_t"], f32)
    lnb_t = np.asarray(inp["lnb_t"], f32)
    mplng = np.asarray(inp["mplng"], f32)
    mplnb = np.asarray(inp["mplnb"], f32)

    # LayerNorm gain/bias of xb/xt/mp_out are folded into every downstream
    # weight (row-scale by g) and bias (b @ W.T); the kernel stores only the
    # normalized core (x - mean) / std.
    def gb(i, which):  # scale/bias produced by the LN feeding layer i input
        if i == 0:
            return np.ones(D, f32), np.zeros(D, f32)
        return (lng_b[i - 1], lnb_b[i - 1]) if which == "b" else \
               (lng_t[i - 1], lnb_t[i - 1])

    wlist, wi = [], {}

    def addW(name, WT):
        wi[name] = len(wlist)
        for kc in range(WT.shape[0] // 128):
            wlist.append(np.ascontiguousarray(WT[kc * 128:(kc + 1) * 128]).astype(BF16))

    for i in range(L):
        g_b, _ = gb(i, "b")
        g_t, _ = gb(i, "t")
        addW(("WtT_b", i), g_b[:, None] * Wt_b[i].T)
        addW(("WcT_b", i), g_b[:, None] * (Wr_b[i] @ Wt_b[i]).T)
        addW(("WctT", i), g_t[:, None] * (Wl_b[i] @ Wt_t[i]).T)
        addW(("WtT_t", i), g_t[:, None] * Wt_t[i].T)
        addW(("WcT_t", i), g_t[:, None] * (Wr_t[i] @ Wt_t[i]).T)
        addW(("WlT_t", i), Wl_t[i].T)
    mpW = np.asarray(inp["mpW"], f32)
    Wp1b = np.asarray(inp["Wp1b"], f32)
    Wp1t = np.asarray(inp["Wp1t"], f32)
    addW("mpWT", lng_t[2][:, None] * mpW.T)
    addW("mpWT_x", lng_b[2][:, None] * (sw * mpW.T))
    addW("Wp1baT", lng_b[2][:, None] * Wp1b[:, :D].T)
    addW("Wp1bbT", mplng[:, None] * Wp1b[:, D:].T)
    addW("Wp1tT", lng_t[2][:, None] * Wp1t.T)
    Wc_pm = _pmajor(wlist)
    NW = len(wlist)

    w128list, w128i = [], {}

    def addW128(name, WT):
        w128i[name] = len(w128list)
        for kc in range(WT.shape[0] // 128):
            w128list.append(np.ascontiguousarray(WT[kc * 128:(kc + 1) * 128]).astype(BF16))

    addW128("Wp2bT", np.asarray(inp["Wp2b"], f32).T)
    addW128("Wp2tT", np.asarray(inp["Wp2t"], f32).T)
    W128_pm = _pmajor(w128list)

    vlist, vi = [], {}

    def addV(name, v):
        vi[name] = len(vlist)
        vlist.append(np.ascontiguousarray(
            np.broadcast_to(v.astype(f32), (128, v.shape[0]))).astype(BF16))

    for i in range(L):
        _, b_b = gb(i, "b")
        _, b_t = gb(i, "t")
        addV(("bt_b", i), bt_b[i] + b_b @ Wt_b[i].T)
        addV(("blc_b", i), bl_b[i] + Wr_b[i] @ bt_b[i]
             + b_b @ (Wr_b[i] @ Wt_b[i]).T)
        addV(("vttl", i), Wl_b[i] @ bt_t[i] + b_t @ (Wl_b[i] @ Wt_t[i]).T)
        addV(("bt_t", i), bt_t[i] + b_t @ Wt_t[i].T)
        addV(("blc_t", i), bl_t[i] + Wr_t[i] @ bt_t[i]
             + b_t @ (Wr_t[i] @ Wt_t[i]).T)
    addV("mpb", np.asarray(inp["mpb"], f32) + lnb_b[2] @ (sw * mpW.T))
    addV("vmpt", lnb_t[2] @ mpW.T)
    addV("bp1b", np.asarray(inp["bp1b"], f32) + lnb_b[2] @ Wp1b[:, :D].T
         + mplnb @ Wp1b[:, D:].T)
    addV("bp1t", np.asarray(inp["bp1t"], f32) + lnb_t[2] @ Wp1t.T)
    for nm in ("plngb", "plnbb", "plngt", "plnbt"):
        addV(nm, np.asarray(inp[nm], f32))
    if len(vlist) % 2:
        addV("_pad", np.zeros(D, f32))
    V_pm = _pmajor(vlist)
    NV = len(vlist)
    rowsA = np.zeros((NC, 1, BP), BF16)
    rsum = A_mp.sum(axis=1).astype(f32)  # [N_B]
    rowsA.reshape(NC, BP)[:, :B_SH] = rsum.reshape(NC, B_SH).astype(BF16)

    v128list, v128i = [], {}
    for nm in ("bp2b", "bp2t"):
        v128i[nm] = len(v128list)
        v128list.append(np.ascontiguousarray(
            np.broadcast_to(np.asarray(inp[nm], f32), (128, 128))).astype(BF16))
    V128_pm = _pmajor(v128list)

    ident = np.eye(128, dtype=f32).astype(BF16)
    temp = float(np.asarray(inp["temperature"]).reshape(-1)[0])
    simb = float(np.asarray(inp["sim_bias"]).reshape(-1)[0])

    shared = dict(xt0=xt0_pm, Wc256=Wc_pm, W128=W128_pm, V256=V_pm,
                  V128=V128_pm, ident=ident)
    in_maps = []
    for c in range(NC):
        m = dict(shared)
        m["xb0"] = _pmajor(xb0[c])
        m["At"] = np.ascontiguousarray(At[c])
        m["Abt"] = np.ascontiguousarray(Abt[c])
        m["Amp"] = np.ascontiguousarray(Amp[c])
        m["rowsA"] = np.ascontiguousarray(rowsA[c])
        m["xtO0"] = np.ascontiguousarray(xtO0[c])
        in_maps.append(m)
    meta = dict(wi=wi, w128i=w128i, vi=vi, v128i=v128i,
                wcount=NW, vcount=NV, temp=temp, simb=simb)
    return in_maps, meta


# ---------------------------------------------------------------------------
# Device program
# ---------------------------------------------------------------------------

def build_program(meta):
    nc = bacc.Bacc("TRN2", target_bir_lowering=False, debug=False,
                   num_devices=NC)
    wi, w128i, vi, v128i = meta["wi"], meta["w128i"], meta["vi"], meta["v128i"]
    NW, NV = meta["wcount"], meta["vcount"]
    temp = meta["temp"]

    xb0_d = nc.dram_tensor("xb0", [128, NBT * D], BF, kind="ExternalInput")
    xt0_d = nc.dram_tensor("xt0", [128, NTT * D], BF, kind="ExternalInput")
    At_d = nc.dram_tensor("At", [NTT, 128, BP], BF, kind="ExternalInput")
    Abt_d = nc.dram_tensor("Abt", [NBT, 128, TP], BF, kind="ExternalInput")
    Amp_d = nc.dram_tensor("Amp", [NTT, 128, BP], BF, kind="ExternalInput")
    Wc_d = nc.dram_tensor("Wc256", [128, NW * D], BF, kind="ExternalInput")
    W128_d = nc.dram_tensor("W128", [128, 4 * 128], BF, kind="ExternalInput")
    V256_d = nc.dram_tensor("V256", [128, NV * D], BF, kind="ExternalInput")
    V128_d = nc.dram_tensor("V128", [128, 2 * 128], BF, kind="ExternalInput")
    id_d = nc.dram_tensor("ident", [128, 128], BF, kind="ExternalInput")
    rowsA_d = nc.dram_tensor("rowsA", [1, BP], BF, kind="ExternalInput")
    xtO0_d = nc.dram_tensor("xtO0", [128, 2, TB], BF, kind="ExternalInput")
    sim_d = nc.dram_tensor("simO", [NBT, 128, TP], F16, kind="ExternalOutput")

    with tile.TileContext(nc) as tc, contextlib.ExitStack() as ctx:
        cpool = ctx.enter_context(tc.tile_pool(name="const", bufs=1))
        fpool = ctx.enter_context(tc.tile_pool(name="feat", bufs=1))
        spool = ctx.enter_context(tc.tile_pool(name="stream", bufs=10))
        bpool = ctx.enter_context(tc.tile_pool(name="abt_stream", bufs=3))
        gpool = ctx.enter_context(tc.tile_pool(name="gstage", bufs=2))
        epool = ctx.enter_context(tc.tile_pool(name="epi", bufs=7))
        qpool = ctx.enter_context(tc.tile_pool(name="sq", bufs=2))
        tpool = ctx.enter_context(tc.tile_pool(name="tiny", bufs=10))
        opool = ctx.enter_context(tc.tile_pool(name="simout", bufs=3))
        dpool = ctx.enter_context(tc.tile_pool(name="dram", bufs=2, space="DRAM"))

        # ---- loads: few huge p-major DMAs; xb/ident first for layer 0 ----
        ident = cpool.tile([128, 128], BF)
        nc.sync.dma_start(ident[:], id_d[:])
        xb = fpool.tile([128, NBT, D], BF, tag="xb")
        for h in range(5):
            nc.sync.dma_start(xb[:, ds(6 * h, 6), :],
                              xb0_d[:, ds(6 * h * D, 6 * D)])
        wc = cpool.tile([128, NW, D], BF)
        assert NW % 2 == 0
        for h in range(2):
            nc.scalar.dma_start(wc[:, ds(NW // 2 * h, NW // 2), :],
                                Wc_d[:, ds(NW // 2 * h * D, NW // 2 * D)])
        vb = cpool.tile([128, NV, D], BF)
        assert NV % 2 == 0
        for h in range(2):
            nc.gpsimd.dma_start(vb[:, ds(NV // 2 * h, NV // 2), :],
                                V256_d[:, ds(NV // 2 * h * D, NV // 2 * D)])
        xt = fpool.tile([128, NTT, D], BF, tag="xt")
        nc.sync.dma_start(xt[:], xt0_d[:])
        w128 = cpool.tile([128, 4, 128], BF)
        nc.gpsimd.dma_start(w128[:], W128_d[:])
        vb128 = cpool.tile([128, 2, 128], BF)
        nc.gpsimd.dma_start(vb128[:], V128_d[:])
        epsb = cpool.tile([128, 1], F32, name="epsb")
        nc.gpsimd.memset(epsb[:], LN_EPS)
        eps24 = cpool.tile([128, 1], F32, name="eps24")
        nc.gpsimd.memset(eps24[:], 1e-24)
        ones1 = cpool.tile([1, 512], BF, name="ones1")
        nc.gpsimd.memset(ones1[:], 1.0)
        rA = cpool.tile([1, BP], BF, name="rowsA")
        nc.gpsimd.dma_start(rA[:], rowsA_d[:])

        W = lambda name, kc: wc[:, wi[name] + kc, :]
        V = lambda name: vb[:, vi[name], :]
        Vrow = lambda name: vb[0:1, vi[name], :]

        xbT = fpool.tile([128, 2, BP], BF, tag="xbT")
        xtT = fpool.tile([128, 2, TP], BF, tag="xtT")
        tb_bf = fpool.tile([128, NBT, D], BF, tag="tb_bf")
        ttl_bf = fpool.tile([128, NTT, D], BF, tag="ttl_bf")
        ptb_bf = fpool.tile([128, 2, TP], BF, tag="ptb_bf")
        pm_sb = fpool.tile([128, 2, TB], BF, tag="pm_sb")
        xtO_T = fpool.tile([128, 2, TB], BF, tag="xtO_T")
        xtnP = fpool.tile([128, 2, D], BF, tag="xtnP")

        def transpose_into(dst, src_tile, n_tiles, trp):
            for nt in range(n_tiles):
                for kc in range(2):
                    ps = trp.tile([128, 128], BF, tag="tr")
                    nc.tensor.transpose(ps[:], src_tile[:, nt, ts(kc, 128)], ident[:])
                    if (2 * nt + kc) % 2 == 0:
                        nc.vector.tensor_copy(dst[:, kc, ts(nt, 128)], ps[:])
                    else:
                        nc.scalar.copy(dst[:, kc, ts(nt, 128)], ps[:])

        def _ln_stats(sb_ap):
            st6 = tpool.tile([128, 6], F32, tag="st6")
            nc.vector.bn_stats(st6[:], sb_ap)
            mv = tpool.tile([128, 2], F32, tag="mv")
            nc.vector.bn_aggr(mv[:], st6[:])
            std = tpool.tile([128, 1], F32, tag="std")
            nc.scalar.activation(std[:], mv[:, 1:2], AF.Sqrt, bias=epsb[:])
            inv = tpool.tile([128, 1], F32, tag="inv")
            nc.vector.reciprocal(inv[:], std[:])
            return mv, inv

        def ln_core(sb_ap, out_ap):
            # gain/bias folded into downstream weights: emit (x - mean)/std
            mv, inv = _ln_stats(sb_ap)
            nc.vector.tensor_scalar(out_ap, sb_ap, mv[:, 0:1], inv[:],
                                    ALU.subtract, ALU.mult)

        def ln_epilogue(sb_ap, g_ap, b_ap, out_ap):
            mv, inv = _ln_stats(sb_ap)
            t1 = epool.tile([128, D], F32, tag="lnt")
            nc.vector.scalar_tensor_tensor(t1[:], sb_ap, mv[:, 0:1], g_ap,
                                           ALU.subtract, ALU.mult)
            nc.vector.scalar_tensor_tensor(out_ap, t1[:], inv[:], b_ap,
                                           ALU.mult, ALU.add)

        def l2_recip(v_ap, width=D, scale=None):
            ssq = tpool.tile([128, 1], F32, tag="l2ssq")
            scr = qpool.tile([128, D], F32, tag="sq")
            nc.scalar.activation(scr[:, :width], v_ap, AF.Square, accum_out=ssq[:])
            nrm = tpool.tile([128, 1], F32, tag="l2n")
            nc.scalar.activation(nrm[:], ssq[:], AF.Sqrt, bias=eps24[:])
            rec = tpool.tile([128, 1], F32, tag="l2r")
            nc.vector.reciprocal(rec[:], nrm[:])
            if scale is not None:
                nc.scalar.mul(rec[:], rec[:], scale)
            return rec

        def dmajor_agg(cbp, trp, adj_d, ntb, nts, wth, stat_tile, wx_name,
                       bias_name, dma_alt, extra=None):
            """cb[dout, b] = sum_t stat[t, dout]^T adj[t, b] (+ Wx-fold + bias)
            for one group of node tiles; returns node-major bf16 PSUM views
            (two tiles packed per PSUM bank tile)."""
            cbT = [cbp.tile([128, 512], F32, tag="cbT", name=f"cb{ntb}_{dh}")
                   for dh in range(2)]
            for tck in range(NTT):
                at = spool.tile([128, 512], BF, tag="at")
                eng = nc.sync if (tck + dma_alt) % 2 == 0 else nc.scalar
                eng.dma_start(at[:, :wth], adj_d[tck][:, ds(ntb * 512, wth)])
                for dh in range(2):
                    nc.tensor.matmul(cbT[dh][:, :wth],
                                     stat_tile[:, tck, ts(dh, 128)],
                                     at[:, :wth], start=tck == 0, stop=False)
            for kc in range(2):
                for dh in range(2):
                    nc.tensor.matmul(cbT[dh][:, :wth],
                                     W(wx_name, kc)[:, ts(dh, 128)],
                                     xbT[:, kc, ds(ntb * 512, wth)],
                                     start=False, stop=False)
            for dh in range(2):
                nc.tensor.matmul(cbT[dh][:, :wth],
                                 vb[0:1, vi[bias_name], ts(dh, 128)],
                                 ones1[0:1, :wth], start=False,
                                 stop=extra is None)
            if extra is not None:
                vname, rrow = extra
                for dh in range(2):
                    nc.tensor.matmul(cbT[dh][:, :wth],
                                     vb[0:1, vi[vname], ts(dh, 128)],
                                     rrow[0:1, ds(ntb * 512, wth)],
                                     start=False, stop=True)
            cbs = gpool.tile([128, 2, 512], BF, tag="cbs")
            nc.vector.tensor_copy(cbs[:, 0, :wth], cbT[0][:, :wth])
            nc.scalar.copy(cbs[:, 1, :wth], cbT[1][:, :wth])
            out = []
            for k, nt in enumerate(nts):
                if k % 2 == 0:
                    trC2 = trp.tile([128, 2, 256], BF, tag="trC")
                for kc in range(2):
                    nc.tensor.transpose(trC2[:, k % 2, ts(kc, 128)],
                                        cbs[:, kc, ts(k, 128)], ident[:])
                out.append(trC2[:, k % 2, :])
            return out

        # ================= layers =================
        for i in range(L):
            # ---- A1: xb transposes + tb (bias folded via k=1 matmul) ----
            with tc.tile_pool(name=f"psA{i}", bufs=4, space="PSUM") as mmA, \
                 tc.tile_pool(name=f"psAt{i}", bufs=2, space="PSUM") as trA:
                transpose_into(xbT, xb, NBT, trA)
                for nt in range(NBT):
                    ps = mmA.tile([128, D], F32, tag="mm")
                    for kc in range(2):
                        nc.tensor.matmul(ps[:], xbT[:, kc, ts(nt, 128)],
                                         W(("WtT_b", i), kc),
                                         start=kc == 0, stop=False)
                    nc.tensor.matmul(ps[:], ones1[0:1, :128],
                                     Vrow(("bt_b", i)), start=False, stop=True)
                    if nt % 2 == 0:
                        nc.vector.tensor_copy(tb_bf[:, nt, :], ps[:])
                    else:
                        nc.scalar.copy(tb_bf[:, nt, :], ps[:])

            rs_in = dpool.tile([NC, 2, 128, TB], BF, tag="rs_in")
            rs_out = dpool.tile([2, 128, TB], BF, tag="rs_out")
            ag_in = dpool.tile([2, 128, D], BF, tag="ag_in")
            ag_out = dpool.tile([NTT, 128, D], BF, tag="ag_out",
                                addr_space="Shared")

            # ---- B: partial_t (d-major) ----
            with tc.tile_pool(name=f"psB{i}", bufs=2, space="PSUM") as ptp:
                pt = [ptp.tile([128, TP], F32, tag="pt", name=f"pt{i}_{dh}")
                      for dh in range(2)]
                for c in range(NBT):
                    ab = bpool.tile([128, TP], BF, tag="abt")
                    nc.sync.dma_start(ab[:], Abt_d[c])
                    for dh in range(2):
                        for s in range(4):
                            nc.tensor.matmul(pt[dh][:, ts(s, 512)],
                                             tb_bf[:, c, ts(dh, 128)],
                                             ab[:, ts(s, 512)],
                                             start=c == 0, stop=c == NBT - 1)
                for dh in range(2):
                    for s in range(4):
                        if s % 2 == 0:
                            nc.vector.tensor_copy(ptb_bf[:, dh, ts(s, 512)],
                                                  pt[dh][:, ts(s, 512)])
                        else:
                            nc.scalar.copy(ptb_bf[:, dh, ts(s, 512)],
                                           pt[dh][:, ts(s, 512)])
            for k in range(NC):
                for dh in range(2):
                    nc.gpsimd.dma_start(rs_in[k, dh], ptb_bf[:, dh, ts(k, TB)])

            # ---- A2: xt transposes + ttl + RS xtT payload ----
            with tc.tile_pool(name=f"psA2{i}", bufs=3, space="PSUM") as mmA2, \
                 tc.tile_pool(name=f"psA2t{i}", bufs=2, space="PSUM") as trA2:
                transpose_into(xtT, xt, NTT, trA2)
                for tt_ in range(NTT):
                    ps = mmA2.tile([128, D], F32, tag="mm")
                    for kc in range(2):
                        nc.tensor.matmul(ps[:], xtT[:, kc, ts(tt_, 128)],
                                         W(("WctT", i), kc),
                                         start=kc == 0, stop=False)
                    nc.tensor.matmul(ps[:], ones1[0:1, :128],
                                     Vrow(("vttl", i)), start=False, stop=True)
                    if tt_ % 2 == 0:
                        nc.vector.tensor_copy(ttl_bf[:, tt_, :], ps[:])
                    else:
                        nc.scalar.copy(ttl_bf[:, tt_, :], ps[:])
            nc.gpsimd.collective_compute(
                "ReduceScatter", ALU.add, replica_groups=[list(range(NC))],
                ins=[rs_in.opt()], outs=[rs_out.opt()])

            # ---- D (emitted mid-C): sharded trait update + AG ----
            def emit_D(mmD, trp):
                # own trait block of xt(i), d-major: input for layer 0,
                # else transpose of the previous trait update (xtnP)
                if i == 0:
                    nc.gpsimd.dma_start(xtO_T[:], xtO0_d[:])
                else:
                    trD = trp.tile([128, 2, 256], BF, tag="trC")
                    for j in range(2):
                        for kc in range(2):
                            nc.tensor.transpose(trD[:, kc, ts(j, 128)],
                                                xtnP[:, j, ts(kc, 128)],
                                                ident[:])
                    nc.vector.tensor_copy(xtO_T[:], trD[:])
                for h in range(2):
                    nc.gpsimd.dma_start(pm_sb[:, h, :], rs_out[h])
                for j in range(2):
                    ps = mmD.tile([128, D], F32, tag="mmD")
                    for kc in range(2):
                        nc.tensor.matmul(ps[:], pm_sb[:, kc, ts(j, 128)],
                                         W(("WlT_t", i), kc),
                                         start=kc == 0, stop=False)
                    for kc in range(2):
                        nc.tensor.matmul(ps[:], xtO_T[:, kc, ts(j, 128)],
                                         W(("WcT_t", i), kc),
                                         start=False, stop=False)
                    nc.tensor.matmul(ps[:], ones1[0:1, :128],
                                     Vrow(("blc_t", i)), start=False, stop=True)
                    pstt = mmD.tile([128, D], F32, tag="mmD")
                    for kc in range(2):
                        nc.tensor.matmul(pstt[:], xtO_T[:, kc, ts(j, 128)],
                                         W(("WtT_t", i), kc),
                                         start=kc == 0, stop=False)
                    nc.tensor.matmul(pstt[:], ones1[0:1, :128],
                                     Vrow(("bt_t", i)), start=False, stop=True)
                    tt_sb = epool.tile([128, D], F32, tag="lnt")
                    nc.scalar.copy(tt_sb[:], pstt[:])
                    rec = l2_recip(ps[:])
                    s1 = epool.tile([128, D], F32, tag="s1")
                    nc.vector.scalar_tensor_tensor(s1[:], ps[:], rec[:],
                                                   tt_sb[:], ALU.mult, ALU.add)
                    ln_core(s1[:], xtnP[:, j, :])
                    nc.gpsimd.dma_start(ag_in[j], xtnP[:, j, :])
                nc.gpsimd.collective_compute(
                    "AllGather", ALU.bypass, replica_groups=[list(range(NC))],
                    ins=[ag_in.opt()], outs=[ag_out.opt()])
                for j in range(NTT):
                    nc.gpsimd.dma_start(xt[:, j, :], ag_out[j])

            # ---- C: d-major bacteria aggregation + update ----
            with tc.tile_pool(name=f"psC{i}", bufs=3, space="PSUM") as cbp, \
                 tc.tile_pool(name=f"psCt{i}", bufs=3, space="PSUM") as trp, \
                 tc.tile_pool(name=f"psD{i}", bufs=2, space="PSUM") as mmD:
                for ntb in range(NG):
                    nts = [ntb * 4 + k for k in range(4) if ntb * 4 + k < NBT]
                    wth = len(nts) * 128
                    trCs = dmajor_agg(cbp, trp, At_d, ntb, nts, wth, ttl_bf,
                                      ("WcT_b", i), ("blc_b", i), 0)
                    for k, nt in enumerate(nts):
                        trC = trCs[k]
                        rec = l2_recip(trC)
                        sb = epool.tile([128, D], F32, tag="sb")
                        nc.vector.scalar_tensor_tensor(sb[:], trC, rec[:],
                                                       tb_bf[:, nt, :],
                                                       ALU.mult, ALU.add)
                        ln_core(sb[:], xb[:, nt, :])
                    if ntb == 6:
                        emit_D(mmD, trp)

        # ================= final =================
        xtm_bf = fpool.tile([128, NTT, D], BF, tag="ttl_bf")
        htn_T = fpool.tile([128, TP], BF, tag="ptb_bf")
        mpT_bf = fpool.tile([128, 2, BP], BF, tag="xb")
        hbT_bf = fpool.tile([128, 2, BP], BF, tag="tb_bf")
        agh_in = dpool.tile([2, 128, 128], BF, tag="agh_in")
        agh_out = dpool.tile([NTT, 128, 128], BF, tag="agh_out",
                             addr_space="Shared")

        # F1: xb/xt transposes, xtm (all tiles), own-trait projection + AG
        with tc.tile_pool(name="psF1", bufs=3, space="PSUM") as mmF, \
             tc.tile_pool(name="psF1s", bufs=2, space="PSUM") as mmFs, \
             tc.tile_pool(name="psF1t", bufs=2, space="PSUM") as trF:
            transpose_into(xbT, xb, NBT, trF)
            transpose_into(xtT, xt, NTT, trF)
            for tt_ in range(NTT):
                ps = mmF.tile([128, D], F32, tag="mm")
                for kc in range(2):
                    nc.tensor.matmul(ps[:], xtT[:, kc, ts(tt_, 128)],
                                     W("mpWT", kc), start=kc == 0, stop=kc == 1)
                if tt_ % 2 == 0:
                    nc.vector.tensor_copy(xtm_bf[:, tt_, :], ps[:])
                else:
                    nc.scalar.copy(xtm_bf[:, tt_, :], ps[:])
            # own trait block: transpose xtnP (layer-2 update, node-major)
            xtO3 = fpool.tile([128, 2, D], BF, tag="pm_sb")
            for j in range(2):
                for kc in range(2):
                    pst = trF.tile([128, 128], BF, tag="tr")
                    nc.tensor.transpose(pst[:], xtnP[:, j, ts(kc, 128)], ident[:])
                    nc.vector.tensor_copy(xtO3[:, kc, ts(j, 128)], pst[:])
            for j in range(2):
                ps = mmF.tile([128, D], F32, tag="mm")
                for kc in range(2):
                    nc.tensor.matmul(ps[:], xtO3[:, kc, ts(j, 128)],
                                     W("Wp1tT", kc), start=kc == 0, stop=False)
                nc.tensor.matmul(ps[:], ones1[0:1, :128], Vrow("bp1t"),
                                 start=False, stop=True)
                lno = epool.tile([128, D], F32, tag="sb")
                ln_epilogue(ps[:], V("plngt"), V("plnbt"), lno[:])
                htr = epool.tile([128, D], BF, tag="relu_bf")
                nc.scalar.activation(htr[:], lno[:], AF.Relu)
                htT = epool.tile([128, 2, 128], BF, tag="htT")
                for kc in range(2):
                    pst = trF.tile([128, 128], BF, tag="tr")
                    nc.tensor.transpose(pst[:], htr[:, ts(kc, 128)], ident[:])
                    if kc == 0:
                        nc.vector.tensor_copy(htT[:, kc, :], pst[:])
                    else:
                        nc.scalar.copy(htT[:, kc, :], pst[:])
                ps2 = mmFs.tile([128, 128], F32, tag="mms")
                for kc in range(2):
                    nc.tensor.matmul(ps2[:], htT[:, kc, :],
                                     w128[:, w128i["Wp2tT"] + kc, :],
                                     start=kc == 0, stop=False)
                nc.tensor.matmul(ps2[:], ones1[0:1, :128],
                                 vb128[0:1, v128i["bp2t"], :],
                                 start=False, stop=True)
                rec = l2_recip(ps2[:], width=128, scale=temp)
                hn = epool.tile([128, 128], BF, tag="h2n")
                nc.scalar.activation(hn[:], ps2[:], AF.Copy, scale=rec[:])
                pst = trF.tile([128, 128], BF, tag="tr")
                nc.tensor.transpose(pst[:], hn[:], ident[:])
                htn = epool.tile([128, 128], BF, tag="htn")
                nc.vector.tensor_copy(htn[:], pst[:])
                nc.gpsimd.dma_start(agh_in[j], htn[:])
        nc.gpsimd.collective_compute(
            "AllGather", ALU.bypass, replica_groups=[list(range(NC))],
            ins=[agh_in.opt()], outs=[agh_out.opt()])
        for j in range(NTT):
            nc.gpsimd.dma_start(htn_T[:, ts(j, 128)], agh_out[j])

        # P1: metapath d-major agg + LN -> mpT_bf
        with tc.tile_pool(name="psP1", bufs=2, space="PSUM") as cbp1, \
             tc.tile_pool(name="psP1t", bufs=4, space="PSUM") as trp1, \
             tc.tile_pool(name="psP1b", bufs=2, space="PSUM") as trb1:
            for ntb in range(NG):
                nts = [ntb * 4 + k for k in range(4) if ntb * 4 + k < NBT]
                wth = len(nts) * 128
                trCs = dmajor_agg(cbp1, trp1, Amp_d, ntb, nts, wth, xtm_bf,
                                  "mpWT_x", "mpb", 1, extra=("vmpt", rA))
                for k, nt in enumerate(nts):
                    mpo = epool.tile([128, D], BF, tag="mpo_bf")
                    ln_core(trCs[k], mpo[:])
                    for kc in range(2):
                        pst = trb1.tile([128, 128], BF, tag="trb")
                        nc.tensor.transpose(pst[:], mpo[:, ts(kc, 128)], ident[:])
                        if kc == 0:
                            nc.vector.tensor_copy(mpT_bf[:, kc, ts(nt, 128)],
                                                  pst[:])
                        else:
                            nc.scalar.copy(mpT_bf[:, kc, ts(nt, 128)], pst[:])

        # P2: proj1 + LN + relu -> hbT_bf
        with tc.tile_pool(name="psP2", bufs=4, space="PSUM") as mmY, \
             tc.tile_pool(name="psP2t", bufs=2, space="PSUM") as trY:
            for nt in range(NBT):
                ps1 = mmY.tile([128, D], F32, tag="mm")
                for kc in range(2):
                    nc.tensor.matmul(ps1[:], xbT[:, kc, ts(nt, 128)],
                                     W("Wp1baT", kc), start=kc == 0,
                                     stop=False)
                for kc in range(2):
                    nc.tensor.matmul(ps1[:], mpT_bf[:, kc, ts(nt, 128)],
                                     W("Wp1bbT", kc), start=False, stop=False)
                nc.tensor.matmul(ps1[:], ones1[0:1, :128], Vrow("bp1b"),
                                 start=False, stop=True)
                lno = epool.tile([128, D], F32, tag="sb")
                ln_epilogue(ps1[:], V("plngb"), V("plnbb"), lno[:])
                hbr = epool.tile([128, D], BF, tag="relu_bf")
                nc.scalar.activation(hbr[:], lno[:], AF.Relu)
                for kc in range(2):
                    pst = trY.tile([128, 128], BF, tag="tr")
                    nc.tensor.transpose(pst[:], hbr[:, ts(kc, 128)], ident[:])
                    if kc == 0:
                        nc.vector.tensor_copy(hbT_bf[:, kc, ts(nt, 128)], pst[:])
                    else:
                        nc.scalar.copy(hbT_bf[:, kc, ts(nt, 128)], pst[:])

        # P3: proj2 + l2 + sim sweep, whole-tile f16 writes (2000 cols)
        SIMW = [512, 512, 512, N_T - 3 * 512]
        with tc.tile_pool(name="psP3", bufs=3, space="PSUM") as mmX, \
             tc.tile_pool(name="psP3t", bufs=2, space="PSUM") as trX, \
             tc.tile_pool(name="psSim", bufs=3, space="PSUM") as mmS:
            for nt in range(NBT):
                ps2 = mmX.tile([128, 128], F32, tag="mm")
                for kc in range(2):
                    nc.tensor.matmul(ps2[:], hbT_bf[:, kc, ts(nt, 128)],
                                     w128[:, w128i["Wp2bT"] + kc, :],
                                     start=kc == 0, stop=False)
                nc.tensor.matmul(ps2[:], ones1[0:1, :128],
                                 vb128[0:1, v128i["bp2b"], :],
                                 start=False, stop=True)
                rec = l2_recip(ps2[:], width=128)
                hn = epool.tile([128, 128], BF, tag="h2n")
                nc.scalar.activation(hn[:], ps2[:], AF.Copy, scale=rec[:])
                pst = trX.tile([128, 128], BF, tag="tr")
                nc.tensor.transpose(pst[:], hn[:], ident[:])
                hbnT = epool.tile([128, 128], BF, tag="hbnT")
                nc.vector.tensor_copy(hbnT[:], pst[:])
                ob = opool.tile([128, 2048], F16, tag="simout")
                for s in range(4):
                    w = SIMW[s]
                    pssim = mmS.tile([128, 512], F32, tag="sim")
                    nc.tensor.matmul(pssim[:, :w], hbnT[:],
                                     htn_T[:, ds(s * 512, w)],
                                     start=True, stop=True)
                    if s % 2 == 0:
                        nc.vector.tensor_copy(ob[:, ds(s * 512, w)],
                                              pssim[:, :w])
                    else:
                        nc.scalar.copy(ob[:, ds(s * 512, w)], pssim[:, :w])
                eng = nc.sync if nt % 2 == 0 else nc.gpsimd
                eng.dma_start(sim_d[nt][:, :N_T], ob[:, :N_T])

    nc.compile()
    return nc


# ---------------------------------------------------------------------------
# Entry point
# ---------------------------------------------------------------------------

def kernel(**inputs):
    in_maps, meta = _prep(inputs)
    nc = build_program(meta)
    res = run_bass_kernel_spmd(nc, in_maps, core_ids=list(range(NC)))
    sim = np.empty((N_B, N_T), np.float32)
    for c in range(NC):
        shard = np.asarray(res.results[c]["simO"]).reshape(BP, TP)
        sim[c * B_SH:(c + 1) * B_SH] = shard[:B_SH, :N_T].astype(np.float32)
    if meta["simb"] != 0.0:
        sim += np.float32(meta["simb"])
    return sim
